# revision 25
# baseline (speedup 1.0000x reference)
"""Multi-head causal self-attention on 8 Trainium2 NeuronCores.

Sharding: 4-way data parallel over batch x 2-way tensor parallel over heads.
Core c handles batch c//2 and head group c%2 (8 of 16 heads). Each core
computes QKV projections for its head group, causal attention, and a partial
output projection (row-split Wo) written in bf16; the host sums the two
partials per batch and adds the bias.

Schedule (all bf16 matmuls, fp32 PSUM):
- Heads are processed in PAIRS (2j, 2j+1). The two K^T.Q score matmuls have
  contraction 64 and run concurrently in PE row-groups 0/64 (confirmed ~4ns
  apart in traces), writing the two halves of one [128,1024] PSUM tile. One
  exp (scalar engine) covers both; diagonal blocks are narrowed to the
  causally-live column range on both the matmul and exp side.
- ctx matmuls trail the S/exp stream by LAG kt steps (software pipeline) and
  narrow causally-dead columns out of diagonal blocks.
- V tiles carry a ones column, so the ctx matmul also yields the softmax
  denominators; each pair is normalized right after its ctx accumulation
  finishes, off the PE critical path.
- Query dim is walked in 512-wide chunks. Per-chunk PE filler is cascaded to
  match each chunk's exp-slack: chunk c runs tb=c's trailing K units (first
  touched at tick 16j+4c) plus tb=c+1's lead units (Q/K0/V, needed at the
  next chunk's start); chunk 3 additionally absorbs the cc<3 out-projections.
  Dummy warmup matmuls on a memset tile cover the startup DMA window and
  pre-warm HAM.
- Endgame: the last pair's denominators/reciprocals are computed per-head
  half and overlap the stream drain; outproj3 partials for tt>=2 drain on
  the idle scalar engine; the ctxT(3,3) normalize runs per 128-col block
  with outproj3_fix(tt) pipelined behind each block.
"""

import numpy as np

B, T, D = 4, 2048, 1024
HEADS = 16
N_CORES = 8
HPC = 8               # heads per core
HD = HPC * 64         # 512, per-core projection width
NT_D = D // 128       # 8 d-tiles
TB = 512              # t-block width for QKV streaming
NT_TB = T // TB       # 4
NT_T = T // 128       # 16 t-tiles
CHUNK = 512           # attention q-chunk width
N_CHUNK = T // CHUNK  # 4
NPAIR = HPC // 2      # 4 head pairs (pair j <-> hdt j)

_NC = None


def _build():
    import concourse.tile as tile
    import concourse.mybir as mybir
    from concourse import bacc
    from contextlib import ExitStack

    F32 = mybir.dt.float32
    BF16 = mybir.dt.bfloat16
    EXP = mybir.ActivationFunctionType.Exp
    COPY = mybir.ActivationFunctionType.Copy

    nc = bacc.Bacc("TRN2", target_bir_lowering=False, debug=False,
                   num_devices=N_CORES)

    xT_ext = nc.dram_tensor("xT", [D, T], BF16, kind="ExternalInput")
    wqT_ext = nc.dram_tensor("wqT", [D, HD], BF16, kind="ExternalInput")
    wkT_ext = nc.dram_tensor("wkT", [D, HD], BF16, kind="ExternalInput")
    wvT_ext = nc.dram_tensor("wvT", [D, HD], BF16, kind="ExternalInput")
    woT_ext = nc.dram_tensor("woT", [HD, D], BF16, kind="ExternalInput")
    mask_ext = nc.dram_tensor("mask", [128, 128], BF16, kind="ExternalInput")
    out_ext = nc.dram_tensor("out", [T, D], BF16, kind="ExternalOutput")

    with tile.TileContext(nc) as tc, ExitStack() as ctx:
        # ---- pools (PSUM: 4 + 2 + 2 = 8 banks) ------------------------
        wqkv_pool = ctx.enter_context(tc.tile_pool(name="wqkv", bufs=1))
        wo_pool = ctx.enter_context(tc.tile_pool(name="wo", bufs=1))
        qk_pool = ctx.enter_context(tc.tile_pool(name="qk", bufs=1))
        v_pool = ctx.enter_context(tc.tile_pool(name="v", bufs=1))
        xt_pool = ctx.enter_context(tc.tile_pool(name="xt", bufs=1))
        small = ctx.enter_context(tc.tile_pool(name="small", bufs=1))
        pt_pool = ctx.enter_context(tc.tile_pool(name="pt", bufs=6))
        ctxT_pool = ctx.enter_context(tc.tile_pool(name="ctxT", bufs=4))
        norm_pool = ctx.enter_context(tc.tile_pool(name="norm", bufs=2))
        out_pool = ctx.enter_context(tc.tile_pool(name="outsb", bufs=4))
        s_ps_pool = ctx.enter_context(
            tc.tile_pool(name="sps", bufs=2, space="PSUM"))
        ctx_ps_pool = ctx.enter_context(
            tc.tile_pool(name="ctxps", bufs=1, space="PSUM"))
        proj_ps = ctx.enter_context(
            tc.tile_pool(name="projps", bufs=2, space="PSUM"))

        # ---- static SBUF tensors --------------------------------------
        QT = [qk_pool.tile([128, T], BF16, tag=f"QT{i}", name=f"QT{i}")
              for i in range(NPAIR)]
        KT = [qk_pool.tile([128, T], BF16, tag=f"KT{i}", name=f"KT{i}")
              for i in range(NPAIR)]
        V = [v_pool.tile([128, HPC * 65], BF16, tag=f"V{i}", name=f"V{i}")
             for i in range(NT_T)]

        w_sb = {}

        def load_weights_wo():
            t_ = wo_pool.tile([128, NPAIR * D], BF16, tag="wo", name="wo")
            for h in range(2):
                nc.sync.dma_start(
                    t_[:, h * 2 * D:(h + 1) * 2 * D].rearrange(
                        "p (d c) -> p d c", c=D),
                    woT_ext[h * 256:(h + 1) * 256, :].rearrange(
                        "(d p) c -> p d c", p=128))
            w_sb["o"] = t_

        xa = {}

        def load_x(tb):
            t_ = xt_pool.tile([128, NT_D * TB], BF16, tag=f"xa{tb}",
                              name=f"xa{tb}")
            for h in range(2):
                nc.sync.dma_start(
                    t_[:, h * 4 * TB:(h + 1) * 4 * TB].rearrange(
                        "p (d c) -> p d c", c=TB),
                    xT_ext[h * 512:(h + 1) * 512,
                           tb * TB:(tb + 1) * TB].rearrange(
                        "(d p) c -> p d c", p=128))
            xa[tb] = t_

        # one QKV "unit" = one accumulation chain (8 matmuls) + drain copy
        def qk_unit(tb, wname, dst, hdt):
            xat = xa[tb]
            ps = proj_ps.tile([128, TB], F32, tag="proj", name="projps")
            for dt in range(NT_D):
                nc.tensor.matmul(
                    ps[:],
                    w_sb[wname][:, dt * HD + hdt * 128:
                                dt * HD + (hdt + 1) * 128],
                    xat[:, dt * TB:(dt + 1) * TB],
                    start=(dt == 0), stop=(dt == NT_D - 1))
            nc.vector.tensor_copy(
                dst[hdt][:, tb * TB:(tb + 1) * TB], ps[:])

        def v_unit(tb, j):
            xat = xa[tb]
            tt = tb * (TB // 128) + j
            ps = proj_ps.tile([128, HD], F32, tag="proj", name="projps")
            for dt in range(NT_D):
                nc.tensor.matmul(
                    ps[:],
                    xat[:, dt * TB + j * 128:dt * TB + (j + 1) * 128],
                    w_sb["v"][:, dt * HD:(dt + 1) * HD],
                    start=(dt == 0), stop=(dt == NT_D - 1))
            v3 = V[tt][:].rearrange("p (h c) -> p h c", c=65)
            nc.vector.memset(v3[:, :, 64:65], 1.0)
            nc.vector.tensor_copy(
                v3[:, :, 0:64],
                ps[:].rearrange("p (h c) -> p h c", c=64))

        def qkv_lead_units(tb):
            # the part of tb's QKV that must land before chunk tb starts:
            # all Q (consumed at tick 0), K head-pair 0 (consumed at tick
            # 4*tb), and all V (consumed by pair 0's early ctx emissions)
            us = []
            for hdt in range(NPAIR):
                us.append(lambda tb=tb, hdt=hdt: qk_unit(tb, "q", QT, hdt))
            us.append(lambda tb=tb: qk_unit(tb, "k", KT, 0))
            for j in range(TB // 128):
                us.append(lambda tb=tb, j=j: v_unit(tb, j))
            return us

        def qkv_trail_units(tb):
            # K head-pairs 1-3 of tb aren't touched until pair j of chunk tb
            # reaches its last 4 kt steps (tick 16*j + 4*tb) — run them
            # inside chunk tb itself, where the exp stream has PE slack
            return [lambda tb=tb, hdt=hdt: qk_unit(tb, "k", KT, hdt)
                    for hdt in range(1, NPAIR)]

        # ---- attention: one head pair x one q-chunk -------------------
        ctxT = {}  # (c % 2, j) -> tile
        LAG = 5    # ctx matmuls trail the S/exp stream by this many kt

        def pair_attention(j, c, filler=None, lag=None):
            q0 = c * CHUNK
            kt_max = 4 * c + 3
            h0, h1 = 2 * j, 2 * j + 1
            lag = LAG if lag is None else lag
            ctx01 = ctx_ps_pool.tile([65, 2 * CHUNK], F32, tag="ctx",
                                     name="ctxps")
            pending = []

            def emit_ctx(kt, pt):
                # columns below the causal diagonal contribute nothing;
                # narrowing the matmul instead of zero-filling pt
                off = max(0, kt * 128 - q0)
                nc.tensor.matmul(
                    ctx01[:, off:CHUNK],
                    V[kt][:, h0 * 65:(h0 + 1) * 65],
                    pt[:, off:CHUNK],
                    start=(kt == 0), stop=(kt == kt_max),
                    skip_group_check=True)
                nc.tensor.matmul(
                    ctx01[:, CHUNK + off:2 * CHUNK],
                    V[kt][:, h1 * 65:(h1 + 1) * 65],
                    pt[:, CHUNK + off:2 * CHUNK],
                    start=(kt == 0), stop=(kt == kt_max),
                    skip_group_check=True)

            for kt in range(kt_max + 1):
                off = max(0, kt * 128 - q0)
                s01 = s_ps_pool.tile([128, 2 * CHUNK], F32, tag="s",
                                     name="sps")
                nc.tensor.matmul(
                    s01[:, off:CHUNK],
                    KT[j][0:64, kt * 128:(kt + 1) * 128],
                    QT[j][0:64, q0 + off:q0 + CHUNK],
                    start=True, stop=True)
                nc.tensor.matmul(
                    s01[:, CHUNK + off:2 * CHUNK],
                    KT[j][64:128, kt * 128:(kt + 1) * 128],
                    QT[j][64:128, q0 + off:q0 + CHUNK],
                    start=True, stop=True)
                pt = pt_pool.tile([128, 2 * CHUNK], BF16, tag="pt", name="pt")
                nc.scalar.activation(pt[:, off:2 * CHUNK],
                                     s01[:, off:2 * CHUNK], EXP, scale=0.125)
                if kt * 128 >= q0:
                    nc.vector.tensor_mul(pt[:, off:off + 128],
                                         pt[:, off:off + 128], mask_sb[:])
                    nc.vector.tensor_mul(
                        pt[:, CHUNK + off:CHUNK + off + 128],
                        pt[:, CHUNK + off:CHUNK + off + 128], mask_sb[:])
                pending.append((kt, pt))
                if len(pending) > lag:
                    emit_ctx(*pending.pop(0))
                if filler is not None:
                    filler.tick()
            if c == 3 and j == 3 and filler is not None:
                # the fix path below consumes opart tiles; make sure every
                # deferred unit (incl. outproj3_partial) is emitted first
                filler.drain()
            for item in pending:
                emit_ctx(*item)

            # normalize pair into bf16 ctxT tile (rows 0:64 = h0, 64:128 = h1)
            ct = ctxT_pool.tile([128, CHUNK], BF16, tag=f"ctxT{j}",
                                name=f"ctxT{j}")
            ctxT[(c, j)] = ct
            den = norm_pool.tile([1, 2 * CHUNK], F32, tag="den", name="den")
            rec = norm_pool.tile([1, 2 * CHUNK], F32, tag="rec", name="rec")
            if c == 3 and j == 3:
                # per-head halves so h0's reciprocal lands sooner, and the
                # DVE queue holds nothing but the normalize chain; the
                # post-stream partials drain on the idle scalar engine and
                # their matmuls cover the copy/recip latency on the PE
                nc.vector.tensor_copy(den[:, 0:CHUNK], ctx01[64:65, 0:CHUNK])
                nc.vector.reciprocal_approx_fast(rec[:, 0:CHUNK],
                                                 den[:, 0:CHUNK])
                nc.vector.tensor_copy(den[:, CHUNK:2 * CHUNK],
                                      ctx01[64:65, CHUNK:2 * CHUNK])
                nc.vector.reciprocal_approx_fast(rec[:, CHUNK:2 * CHUNK],
                                                 den[:, CHUNK:2 * CHUNK])
                outproj3_partial(2, drain_on_act=True)
                outproj3_partial(3, drain_on_act=True)
            else:
                nc.vector.tensor_copy(den[:], ctx01[64:65, :])
                nc.vector.reciprocal_approx_fast(rec[:], den[:])
            if c == 3 and j == 3:
                # tail-critical pair: normalize in 128-col blocks so each
                # outproj3_fix(tt) can start as soon as its block is ready
                for tt in range(4):
                    sl = slice(tt * 128, (tt + 1) * 128)
                    bc0 = norm_pool.tile([128, 128], F32, tag=f"bcf0{tt}",
                                         name=f"bcf0{tt}")
                    bc1 = norm_pool.tile([128, 128], F32, tag=f"bcf1{tt}",
                                         name=f"bcf1{tt}")
                    nc.gpsimd.partition_broadcast(
                        bc0[:], rec[0:1, tt * 128:(tt + 1) * 128])
                    nc.gpsimd.partition_broadcast(
                        bc1[:], rec[0:1, CHUNK + tt * 128:
                                      CHUNK + (tt + 1) * 128])
                    nc.vector.tensor_mul(ct[0:64, sl], ctx01[0:64, sl],
                                         bc0[0:64, :])
                    nc.vector.tensor_mul(
                        ct[64:128, sl],
                        ctx01[0:64, CHUNK + tt * 128:CHUNK + (tt + 1) * 128],
                        bc1[64:128, :])
                    outproj3_fix(tt)
            else:
                bc0 = norm_pool.tile([128, CHUNK], F32, tag="bc0", name="bc0")
                bc1 = norm_pool.tile([128, CHUNK], F32, tag="bc1", name="bc1")
                nc.gpsimd.partition_broadcast(bc0[:], rec[0:1, 0:CHUNK])
                nc.gpsimd.partition_broadcast(bc1[:], rec[0:1,
                                                          CHUNK:2 * CHUNK])
                nc.vector.tensor_mul(ct[0:64, :], ctx01[0:64, 0:CHUNK],
                                     bc0[0:64, :])
                nc.vector.tensor_mul(ct[64:128, :],
                                     ctx01[0:64, CHUNK:2 * CHUNK],
                                     bc1[64:128, :])

        def outproj_unit(c, tt):
            q0 = c * CHUNK
            osb = out_pool.tile([128, D], BF16, tag="osb", name="osb")
            for ob in range(D // 512):
                ps = proj_ps.tile([128, 512], F32, tag="proj", name="projps")
                for hdt in range(NPAIR):
                    nc.tensor.matmul(
                        ps[:],
                        ctxT[(c, hdt)][:, tt * 128:(tt + 1) * 128],
                        w_sb["o"][:, hdt * D + ob * 512:
                                  hdt * D + (ob + 1) * 512],
                        start=(hdt == 0), stop=(hdt == NPAIR - 1))
                nc.vector.tensor_copy(
                    osb[:, ob * 512:(ob + 1) * 512], ps[:])
            nc.sync.dma_start(
                out_ext[q0 + tt * 128:q0 + (tt + 1) * 128, :], osb[:])

        # the final chunk's out-projection is split so only the hdt=3 term
        # waits on the last attention pair: the hdt 0-2 partials (+bias) are
        # emitted after that pair and execute during its exp/normalize drain
        opart = {}

        def outproj3_partial(tt, drain_on_act=False):
            for ob in range(D // 512):
                ps = proj_ps.tile([128, 512], F32, tag="proj", name="projps")
                for hdt in range(NPAIR - 1):
                    nc.tensor.matmul(
                        ps[:],
                        ctxT[(3, hdt)][:, tt * 128:(tt + 1) * 128],
                        w_sb["o"][:, hdt * D + ob * 512:
                                  hdt * D + (ob + 1) * 512],
                        start=(hdt == 0), stop=(hdt == NPAIR - 2))
                op = out_pool.tile([128, 512], F32, tag=f"opart{tt}{ob}",
                                   name=f"opart{tt}{ob}", bufs=1)
                opart[(tt, ob)] = op
                if drain_on_act:
                    # post-stream: the scalar engine is idle and keeps the
                    # DVE free for the normalize/fix chain
                    nc.scalar.activation(op[:], ps[:], COPY)
                else:
                    nc.vector.tensor_copy(op[:], ps[:])

        def outproj3_fix(tt):
            q0 = 3 * CHUNK
            osb = out_pool.tile([128, D], BF16, tag="osb", name="osb")
            for ob in range(D // 512):
                ps = proj_ps.tile([128, 512], F32, tag="proj", name="projps")
                nc.tensor.matmul(
                    ps[:],
                    ctxT[(3, 3)][:, tt * 128:(tt + 1) * 128],
                    w_sb["o"][:, 3 * D + ob * 512:3 * D + (ob + 1) * 512],
                    start=True, stop=True)
                nc.vector.tensor_add(osb[:, ob * 512:(ob + 1) * 512],
                                     ps[:], opart[(tt, ob)][:])
            nc.sync.dma_start(
                out_ext[q0 + tt * 128:q0 + (tt + 1) * 128, :], osb[:])

        # ---- emission schedule ----------------------------------------
        # interleave wq and x(tb0) DMAs so the first q-chain can start
        # as soon as possible; everything else follows
        # dummy matmuls on a memset tile (no DMA dependency — starts the
        # instant the engines boot) keep the PE active through the startup
        # DMA window so HAM un-throttles before the first real chain
        mask_sb = small.tile([128, 128], BF16, tag="mask")
        warm_sb = small.tile([128, 128], BF16, tag="warm")
        nc.vector.memset(warm_sb[:], 0.125)
        warm = proj_ps.tile([128, 512], F32, tag="proj", name="projps")
        for _ in range(60):
            nc.tensor.matmul(warm[0:128, 0:128], warm_sb[:], warm_sb[:],
                             start=True, stop=True)
        def load_w(name, ext):
            t_ = wqkv_pool.tile([128, NT_D * HD], BF16, tag=f"w{name}",
                                name=f"w{name}")
            for h in range(2):
                nc.sync.dma_start(
                    t_[:, h * 4 * HD:(h + 1) * 4 * HD].rearrange(
                        "p (d c) -> p d c", c=HD),
                    ext[h * 512:(h + 1) * 512, :].rearrange(
                        "(d p) c -> p d c", p=128))
            w_sb[name] = t_

        # quarter-granularity interleaved wq/x0 loads: spreads the
        # startup-critical 2MB across DMA queues and lands the first
        # chain's slices earliest
        wq_t = wqkv_pool.tile([128, NT_D * HD], BF16, tag="wq", name="wq")
        w_sb["q"] = wq_t
        x0_t = xt_pool.tile([128, NT_D * TB], BF16, tag="xa0", name="xa0")
        xa[0] = x0_t
        for qq in range(4):
            nc.sync.dma_start(
                wq_t[:, qq * 2 * HD:(qq + 1) * 2 * HD].rearrange(
                    "p (d c) -> p d c", c=HD),
                wqT_ext[qq * 256:(qq + 1) * 256, :].rearrange(
                    "(d p) c -> p d c", p=128))
            nc.sync.dma_start(
                x0_t[:, qq * 2 * TB:(qq + 1) * 2 * TB].rearrange(
                    "p (d c) -> p d c", c=TB),
                xT_ext[qq * 256:(qq + 1) * 256, 0:TB].rearrange(
                    "(d p) c -> p d c", p=128))
        nc.sync.dma_start(mask_sb[:], mask_ext[:])
        load_w("k", wkT_ext)
        load_w("v", wvT_ext)
        load_x(1)
        load_weights_wo()

        # tb0 prefix ordered by DMA arrival: all four q-chains touch only
        # wq+x0 (first in the DMA queue), so the PE never idles waiting for
        # wk/wv while they stream in behind
        for hdt in range(NPAIR):
            qk_unit(0, "q", QT, hdt)
        qk_unit(0, "k", KT, 0)
        for jj in range(TB // 128):
            v_unit(0, jj)

        class Filler:
            """Dispense filler units across the chunk's kt stream."""

            def __init__(self, units, total_kt):
                self.units = list(units)
                self.acc = 0.0
                self.rate = len(self.units) / max(1, total_kt)

            def burst(self, n):
                for u in self.units[:n]:
                    u()
                del self.units[:n]

            def tick(self):
                self.acc += self.rate
                while self.acc >= 1.0 and self.units:
                    self.acc -= 1.0
                    self.units.pop(0)()

            def drain(self):
                for u in self.units:
                    u()
                self.units = []

        tb0_rest = []
        for hdt in range(1, NPAIR):
            tb0_rest.append(lambda hdt=hdt: qk_unit(0, "k", KT, hdt))

        # per-chunk filler supply; all out-projections are deferred to the
        # final chunk, whose causal kt range is longest but has no QKV left
        load_x(2)
        load_x(3)
        for c in range(N_CHUNK - 1):
            units = []
            if c > 0:
                units += qkv_trail_units(c)
            if c == 0:
                units += tb0_rest
            units += qkv_lead_units(c + 1)
            fill = Filler(units, NPAIR * (4 * c + 4))
            for j in range(NPAIR):
                if j > 0:
                    fill.burst(2)
                pair_attention(j, c, fill)
            fill.drain()
        # chunk 3: pairs 0-2 consume tb3's trailing K units and the cc<3
        # out-projections; pair 3 gets its own filler of outproj3_partial
        # units (they read ctxT(3, 0..2), so emitting them any earlier would
        # wedge the strict-FIFO PE queue behind an unwritten ctxT tile)
        fill = Filler(qkv_trail_units(3) +
                      [lambda tt=tt, cc=cc: outproj_unit(cc, tt)
                       for cc in range(3) for tt in range(4)], 3 * 16)
        for j in range(3):
            if j > 0:
                fill.burst(3)
            pair_attention(j, 3, fill)
        fill.drain()
        fill3 = Filler([lambda tt=tt: outproj3_partial(tt)
                        for tt in range(2)], 16)
        pair_attention(3, 3, fill3, lag=2)

    nc.compile()
    return nc


def _get_nc():
    global _NC
    if _NC is None:
        _NC = _build()
    return _NC


def _make_in_maps(x, Wq, Wk, Wv, Wo, bo):
    import ml_dtypes
    mask = np.triu(np.ones((128, 128), dtype=np.float32)).astype(
        ml_dtypes.bfloat16)
    xT = [np.ascontiguousarray(x[b].T) for b in range(B)]
    in_maps = []
    for c in range(N_CORES):
        b, g = c // 2, c % 2
        sl = slice(g * HD, (g + 1) * HD)
        in_maps.append({
            "xT": xT[b].astype(ml_dtypes.bfloat16),
            "wqT": np.ascontiguousarray(Wq[sl, :].T).astype(ml_dtypes.bfloat16),
            "wkT": np.ascontiguousarray(Wk[sl, :].T).astype(ml_dtypes.bfloat16),
            "wvT": np.ascontiguousarray(Wv[sl, :].T).astype(ml_dtypes.bfloat16),
            "woT": np.ascontiguousarray(Wo[:, sl].T).astype(ml_dtypes.bfloat16),
            "mask": mask,
        })
    return in_maps


def kernel(x, Wq, Wk, Wv, Wo, bo):
    from concourse.bass_utils import run_bass_kernel_spmd

    x = np.asarray(x, dtype=np.float32)
    Wq = np.asarray(Wq, dtype=np.float32)
    Wk = np.asarray(Wk, dtype=np.float32)
    Wv = np.asarray(Wv, dtype=np.float32)
    Wo = np.asarray(Wo, dtype=np.float32)
    bo = np.asarray(bo, dtype=np.float32)

    nc = _get_nc()
    in_maps = _make_in_maps(x, Wq, Wk, Wv, Wo, bo)
    res = run_bass_kernel_spmd(nc, in_maps, list(range(N_CORES)))
    outs = [np.asarray(res.results[c]["out"], dtype=np.float32)
            for c in range(N_CORES)]
    return np.stack([outs[2 * b] + outs[2 * b + 1] + bo for b in range(B)],
                    axis=0)

